# revision 1
# baseline (speedup 1.0000x reference)
"""Data-parallel cross-entropy loss on 8 Trainium2 NeuronCores (Bass/Tile).

Problem: labels [4096, 50257] f32, truth [4096] int. Output: scalar f32
  mean_i( logsumexp(labels[i]) - labels[i, truth[i]] )

Sharding (data parallel per the hint): batch 4096 -> 8 cores x 512 rows.
Each core:
  - streams its [512, 50257] f32 shard HBM->SBUF in [128, 8192] chunks,
  - ACT engine computes exp() with the fused per-partition accumulate
    output (accum_out), giving per-row chunk sums in one instruction
    (no max subtraction needed: inputs ~N(0,1), exp is in fp32 range),
  - DVE reduces chunk sums, ACT Ln() gives per-row logsumexp,
  - labels[i, truth[i]] is gathered with one indirect DMA per row-block
    (flat element index p*V + truth, block base via element_offset),
  - (lse - picked) is reduced over rows (DVE) then partitions (PE matmul
    against ones) into a [1,1] partial sum written to DRAM.
Host: the all-reduce step -- sum the 8 partials, divide by 4096.
"""

import os
import numpy as np

B, V = 4096, 50257
N_CORES = 8
R = B // N_CORES            # 512 rows per core
P = 128                     # SBUF partitions
NBLK = R // P               # 4 row blocks per core
CHUNK = 8192                # vocab chunk (f32 elements per partition)
CHUNKS = [(c, min(CHUNK, V - c)) for c in range(0, V, CHUNK)]
NCHUNK = len(CHUNKS)        # 7
# last-streamed block drains the ACT pipeline: finish with small pieces so
# the final exp lags the final DMA by ~2us instead of ~7us
TAIL_CHUNKS = [(c, min(CHUNK, V - c)) for c in range(0, 5 * CHUNK, CHUNK)] + [
    (c, min(2048, V - c)) for c in range(5 * CHUNK, V, 2048)
]
BLK_CHUNKS = [CHUNKS] * (NBLK - 1) + [TAIL_CHUNKS]
ACC_COLS = [0]
for _bc in BLK_CHUNKS:
    ACC_COLS.append(ACC_COLS[-1] + len(_bc))

_cache = {}


def _build():
    import concourse.bacc as bacc
    import concourse.bass as bass
    import concourse.tile as tile
    from concourse import mybir

    f32 = mybir.dt.float32
    i32 = mybir.dt.int32

    nc = bacc.Bacc("TRN2", target_bir_lowering=False, debug=False)
    # labels declared flat so the indirect gather can index it elementwise
    labels = nc.dram_tensor("labels", [R * V, 1], f32, kind="ExternalInput")
    truth = nc.dram_tensor("truth", [R, 1], i32, kind="ExternalInput")
    out = nc.dram_tensor("out", [1, 1], f32, kind="ExternalOutput")

    with tile.TileContext(nc) as tc:
        with (
            tc.tile_pool(name="inp", bufs=4) as inp,
            tc.tile_pool(name="stat", bufs=1) as stat,
            tc.tile_pool(name="psum", bufs=1, space="PSUM") as psum,
        ):
            truth_t = stat.tile([P, NBLK], i32)
            iota_t = stat.tile([P, 1], i32)
            idx_t = stat.tile([P, NBLK], i32)
            picked_t = stat.tile([P, NBLK], f32)
            acc_t = stat.tile([P, ACC_COLS[-1]], f32)
            sums_t = stat.tile([P, NBLK], f32)
            lse_t = stat.tile([P, NBLK], f32)
            diff_t = stat.tile([P, NBLK], f32)
            rows_t = stat.tile([P, 1], f32)
            ones_t = stat.tile([P, 1], f32)
            res_t = stat.tile([1, 1], f32)
            scratch_t = stat.tile([P, CHUNK], f32)

            def emit_chunk(b, ci, c0, cw):
                xt = inp.tile([P, CHUNK], f32, tag="xt", name=f"xt{b}_{ci}")
                nc.sync.dma_start(
                    out=xt[:, :cw],
                    in_=bass.AP(labels, b * P * V + c0, [[V, P], [1, cw]]),
                )
                k = ACC_COLS[b] + ci
                nc.scalar.activation(
                    out=scratch_t[:, :cw],
                    in_=xt[:, :cw],
                    func=mybir.ActivationFunctionType.Exp,
                    accum_out=acc_t[:, k : k + 1],
                )

            # get the first big stream DMA in flight before any setup work
            emit_chunk(0, 0, *BLK_CHUNKS[0][0])

            # truth[b*128 + p] viewed as [p, b]
            nc.sync.dma_start(
                out=truth_t[:], in_=bass.AP(truth, 0, [[1, P], [P, NBLK]])
            )
            # per-partition flat base index p*V (int32, < 2^24 so the DVE
            # fp32 ALU keeps it exact)
            nc.gpsimd.iota(iota_t[:], pattern=[[0, 1]], base=0, channel_multiplier=V)
            nc.vector.memset(ones_t[:], 1.0)

            # gather picked[p, b] = labels[(b*128+p)*V + truth[b*128+p]]
            for b in range(NBLK):
                nc.vector.tensor_tensor(
                    out=idx_t[:, b : b + 1],
                    in0=iota_t[:],
                    in1=truth_t[:, b : b + 1],
                    op=mybir.AluOpType.add,
                )
                nc.gpsimd.indirect_dma_start(
                    out=picked_t[:, b : b + 1],
                    out_offset=None,
                    in_=labels.ap(),
                    in_offset=bass.IndirectOffsetOnAxis(
                        ap=idx_t[:, b : b + 1], axis=0
                    ),
                    element_offset=b * P * V,
                )

            # main stream: exp + per-row accumulate
            for b in range(NBLK):
                for ci, (c0, cw) in enumerate(BLK_CHUNKS[b]):
                    if b == 0 and ci == 0:
                        continue
                    emit_chunk(b, ci, c0, cw)

            for b in range(NBLK):
                nc.vector.reduce_sum(
                    out=sums_t[:, b : b + 1],
                    in_=acc_t[:, ACC_COLS[b] : ACC_COLS[b + 1]],
                    axis=mybir.AxisListType.X,
                )

            nc.scalar.activation(
                out=lse_t[:], in_=sums_t[:], func=mybir.ActivationFunctionType.Ln
            )
            nc.vector.tensor_sub(diff_t[:], lse_t[:], picked_t[:])
            nc.vector.reduce_sum(
                out=rows_t[:], in_=diff_t[:], axis=mybir.AxisListType.X
            )

            # partition reduce: [1,1] = rows^T @ ones
            ps_t = psum.tile([1, 1], f32, space="PSUM")
            nc.tensor.matmul(
                out=ps_t[:], lhsT=rows_t[:], rhs=ones_t[:], start=True, stop=True
            )
            nc.vector.tensor_copy(out=res_t[:], in_=ps_t[:])
            nc.sync.dma_start(out=out.ap(), in_=res_t[:])

    nc.compile()
    return nc


def _get_nc():
    if "nc" not in _cache:
        _cache["nc"] = _build()
    return _cache["nc"]


def _shard(labels, truth):
    labels = np.ascontiguousarray(np.asarray(labels), dtype=np.float32).reshape(B, V)
    truth = np.ascontiguousarray(np.asarray(truth)).astype(np.int32).reshape(B)
    in_maps = []
    for c in range(N_CORES):
        lab = labels[c * R : (c + 1) * R].reshape(R * V, 1)
        tr = truth[c * R : (c + 1) * R].reshape(R, 1)
        in_maps.append({"labels": lab, "truth": tr})
    return in_maps


def kernel(labels, truth):
    from concourse.bass_utils import run_bass_kernel_spmd

    nc = _get_nc()
    in_maps = _shard(labels, truth)
    trace = os.environ.get("CE_KERNEL_TRACE", "0") == "1"
    try:
        res = run_bass_kernel_spmd(
            nc, in_maps, core_ids=list(range(N_CORES)), trace=trace
        )
    except ModuleNotFoundError:
        # tracing requested but this container lacks the NTFF profile hook
        # (antenv.axon_hooks); rerun untraced
        os.environ["BASS_NEVER_TRACE"] = "1"
        res = run_bass_kernel_spmd(
            nc, in_maps, core_ids=list(range(N_CORES)), trace=False
        )
    _cache["last_result"] = res
    partials = np.array(
        [res.results[c]["out"][0, 0] for c in range(N_CORES)], dtype=np.float64
    )
    return np.float32(partials.sum() / B)

